# revision 13
# baseline (speedup 1.0000x reference)
"""Causal self-attention with RoPE on 8 Trainium2 NeuronCores.

Problem (hardcoded): x [2, 2048, 1024] f32, w_qkv [1024, 3072], w_out [1024, 1024],
16 heads x head_dim 64, RoPE base 10000, causal softmax, out = attn @ w_out.

Sharding: DP over batch (2) x TP over head-groups (4 heads/core) = 8 cores.
Each core computes QKV for its 4 heads, full causal attention, and a partial
output projection against its 256 rows of w_out. Host sums the 4 partials per
batch element.

Design (cost-model driven, all matmuls bf16 = 1 cyc/row):
  - RoPE via algebraic identity: roped = cosq + tan * rot(cosq), where
    cos/sin are half-symmetric so rot(cos*q) = cos*rot(q).  rot() is a +-1
    permutation matmul on PE (no shuffle DMAs).  The cos-multiply doubles as
    the PSUM->SBUF eviction of the projection.
  - Head 0 runs q-major, fused with phase 1: after each 512-wide projection
    chunk lands, the attention chunks it unlocks run immediately, so the
    Activation engine (exp) starts ~8us in instead of waiting for all of
    phase 1.  Heads 1-3 run strip-major (fewer, wider exp ops); remaining
    phase-1 work (Q/K for heads 2-3) interleaves into head 1's k-loop.
  - Exact-width score strips starting at the k-tile boundary kt*128
    (causal), so the mask shrinks to one [128,128] triangle multiply per
    k-tile.
  - P@V accumulates into per-qc PSUM tiles [65,512]; row 64 is the softmax
    denominator via a ones-column in V.
  - Output projection per q-chunk interleaves into head 3's k-loop; the
    PSUM->SBUF bounce alternates DVE/ACT and the store DMA is split across
    both DMA queues to shorten the tail.
"""
import numpy as np
import ml_dtypes

import concourse.bacc as bacc
import concourse.tile as tile
from concourse import mybir
from concourse.bass_utils import run_bass_kernel_spmd

F32 = mybir.dt.float32
BF16 = mybir.dt.bfloat16
EXP = mybir.ActivationFunctionType.Exp

NP_BF16 = ml_dtypes.bfloat16

B, S, D = 2, 2048, 1024
H, HD = 16, 64
HPC = 4              # heads per core
CV = HPC * HD        # 256 v channels per core
NKT = S // 128       # 16 k-tiles
NSC = S // 512       # 4 seq chunks
SCALE = 1.0 / np.sqrt(HD)
ROPE_BASE = 10000.0


def _build_nc():
    nc = bacc.Bacc(None, target_bir_lowering=False, debug=False)

    xb8 = nc.declare_dram_parameter("xb8", [4, 128, 8, 512], BF16, isOutput=False)
    wqkb = nc.declare_dram_parameter("wqkb", [128, 8, 512], BF16, isOutput=False)
    wvb = nc.declare_dram_parameter("wvb", [128, 8, 256], BF16, isOutput=False)
    wob = nc.declare_dram_parameter("wob", [128, 2, D], BF16, isOutput=False)
    cosb = nc.declare_dram_parameter("cosb", [128, S], BF16, isOutput=False)
    tanb = nc.declare_dram_parameter("tanb", [128, S], BF16, isOutput=False)
    permb = nc.declare_dram_parameter("permb", [128, 128], BF16, isOutput=False)
    utrib = nc.declare_dram_parameter("utrib", [128, 128], BF16, isOutput=False)
    out = nc.declare_dram_parameter("out", [S, D], F32, isOutput=True)

    with tile.TileContext(nc) as tc:
        with (
            tc.tile_pool(name="const", bufs=1) as const,
            tc.tile_pool(name="qkt", bufs=1) as qkt_pool,
            tc.tile_pool(name="vsb", bufs=1) as vsb_pool,
            tc.tile_pool(name="pt", bufs=4) as pt_pool,
            tc.tile_pool(name="rope", bufs=2) as rope_pool,
            tc.tile_pool(name="attn", bufs=1) as attn_pool,
            tc.tile_pool(name="nrm", bufs=3) as nrm,
            tc.tile_pool(name="outp", bufs=2) as outp,
            tc.tile_pool(name="ps", bufs=1, space="PSUM") as ps,
        ):
            # ---- constants / inputs ----
            cos_sb = const.tile([128, S], BF16, name="cos")
            tan_sb = const.tile([128, S], BF16, name="tan")
            perm_sb = const.tile([128, 128], BF16, name="perm")
            utri_sb = const.tile([128, 128], BF16, name="utri")
            wqk_sb = const.tile([128, 8, 512], BF16, name="wqk")
            wv_sb = const.tile([128, 8, 256], BF16, name="wv")
            wo_sb = const.tile([128, 2, D], BF16, name="wo")
            xp_sb = const.tile([128, 8, 4, 512], BF16, name="xp")

            # DMA queues: scalar (ACT) handles small tables + wqk before the
            # first exp arrives; x chunks split across sync/gpsimd by seq.
            nc.scalar.dma_start(out=perm_sb, in_=permb[:, :])
            nc.scalar.dma_start(out=cos_sb, in_=cosb[:, :])
            nc.scalar.dma_start(out=wqk_sb, in_=wqkb[:, :, :])
            nc.scalar.dma_start(out=tan_sb, in_=tanb[:, :])
            nc.scalar.dma_start(out=utri_sb, in_=utrib[:, :])
            for sc in range(4):
                eng = (nc.sync, nc.gpsimd)[sc % 2]
                eng.dma_start(out=xp_sb[:, :, sc, :], in_=xb8[sc, :, :, :])
            nc.gpsimd.dma_start(out=wv_sb, in_=wvb[:, :, :])
            nc.sync.dma_start(out=wo_sb, in_=wob[:, :, :])

            # persistent phase-1 outputs
            QKT = [qkt_pool.tile([128, S], BF16, name=f"qkt{t}") for t in range(4)]
            Vsb = [vsb_pool.tile([128, HPC, 65], BF16, name=f"v{k}") for k in range(NKT)]

            # attnT[qc]: [chan 128, ct 2, q 512], written by normalize,
            # read by the output projection
            attnT = [
                attn_pool.tile([128, 2, 512], BF16, name=f"attnT{qc}")
                for qc in range(NSC)
            ]

            def qk_chunk(ct, sc):
                """Project q/k channel-tile ct for seq chunk sc, fold cos, and
                rope the chunk (perm matmul + tan-mul + add)."""
                sl = slice(sc * 512, (sc + 1) * 512)
                qp = ps.tile([128, 512], F32, tag="strip", bufs=2, name=f"qk{ct}_{sc}")
                for d in range(8):
                    nc.tensor.matmul(
                        qp,
                        wqk_sb[:, d, ct * 128 : (ct + 1) * 128],
                        xp_sb[:, d, sc, :],
                        start=(d == 0),
                        stop=(d == 7),
                    )
                nc.vector.tensor_mul(QKT[ct][:, sl], qp, cos_sb[:, sl])
                rot = ps.tile([128, 512], F32, tag="strip", bufs=2, name=f"rot{ct}_{sc}")
                nc.tensor.matmul(rot, perm_sb, QKT[ct][:, sl], start=True, stop=True)
                tmp = rope_pool.tile([128, 512], BF16, name="ropetmp")
                nc.vector.tensor_mul(tmp, rot, tan_sb[:, sl])
                nc.gpsimd.tensor_add(QKT[ct][:, sl], QKT[ct][:, sl], tmp)

            def v_group(st):
                """Project v for seq tile st (128 positions, all 4 heads)."""
                vp = ps.tile([128, 4, 64], F32, tag="strip", bufs=2, name=f"vps{st}")
                for d in range(8):
                    nc.tensor.matmul(
                        vp,
                        xp_sb[:, d, st // 4, (st % 4) * 128 : (st % 4) * 128 + 128],
                        wv_sb[:, d, :],
                        start=(d == 0),
                        stop=(d == 7),
                    )
                nc.vector.tensor_copy(Vsb[st][:, :, 0:64], vp)
                nc.vector.memset(Vsb[st][:, :, 64:65], 1.0)

            def normalize(h, qc, outT):
                hh = h % 2
                rc = nrm.tile([1, 512], F32, name="rc")
                nc.vector.reciprocal(rc, outT[64:65, :])
                bc = nrm.tile([64, 512], F32, name="bc")
                nc.gpsimd.partition_broadcast(bc, rc)
                nc.vector.tensor_mul(
                    attnT[qc][hh * 64 : hh * 64 + 64, h // 2, :],
                    outT[0:64, :],
                    bc,
                )

            def head0_qmajor():
                """Head 0 q-major, fused with its phase-1 producers."""
                h, hh = 0, 0
                qt, kt_t = QKT[0], QKT[2]
                for qc in range(NSC):
                    qk_chunk(2, qc)
                    qk_chunk(0, qc)
                    for st in range(qc * 4, qc * 4 + 4):
                        v_group(st)
                    outT = ps.tile([128, 512], F32, tag="outT", bufs=4, name=f"o0_{qc}")
                    for kt in range(qc * 4 + 4):
                        r = kt % 4
                        o0 = r * 128 if kt // 4 == qc else 0
                        ksl = slice(kt * 128, (kt + 1) * 128)
                        sps = ps.tile(
                            [128, 512], F32, tag="strip", bufs=2, name=f"s0_{kt}_{qc}"
                        )
                        nc.tensor.matmul(
                            sps[:, o0:512],
                            kt_t[0:64, ksl],
                            qt[0:64, qc * 512 + o0 : (qc + 1) * 512],
                            start=True,
                            stop=True,
                        )
                        p_t = pt_pool.tile([128, 512], BF16, name="p_t")
                        nc.scalar.activation(p_t[:, o0:512], sps[:, o0:512], EXP, scale=SCALE)
                        if kt // 4 == qc:
                            nc.vector.tensor_mul(
                                p_t[:, o0 : o0 + 128], p_t[:, o0 : o0 + 128], utri_sb
                            )
                        nc.tensor.matmul(
                            outT[0:65, o0:512],
                            Vsb[kt][:, h, :],
                            p_t[:, o0:512],
                            start=(kt == 0),
                            stop=(kt == qc * 4 + 3),
                        )
                    normalize(h, qc, outT)

            def attn_head(h, pre=None, post=None):
                """Strip-major causal attention for head h (QKT fully roped).
                `pre`/`post` map kt -> thunks emitted before scores / after the
                normalize of that iteration."""
                qt = QKT[h // 2]
                kt_t = QKT[2 + h // 2]
                hh = h % 2
                outT = [
                    ps.tile([128, 512], F32, tag="outT", bufs=4, name=f"o{h}_{qc}")
                    for qc in range(NSC)
                ]
                for kt in range(NKT):
                    if pre is not None:
                        for thunk in pre.get(kt, ()):
                            thunk()
                    qc0, r = kt // 4, kt % 4
                    ksl = slice(kt * 128, (kt + 1) * 128)
                    # scores strips: cover q in [kt*128, 2048) using 1024-wide
                    # PSUM tiles anchored at qc0*512
                    pts = []
                    for half in range(2):
                        base = qc0 * 512 + half * 1024
                        if base >= S:
                            break
                        wid = min(1024, S - base)
                        sps = ps.tile(
                            [128, 1024], F32, tag="strip", bufs=2, name=f"s{h}_{kt}_{half}"
                        )
                        o0 = r * 128 if half == 0 else 0
                        for j in range(0, wid, 512):
                            co = max(o0, j)
                            ce = min(j + 512, wid)
                            if co >= ce:
                                continue
                            nc.tensor.matmul(
                                sps[:, co:ce],
                                kt_t[hh * 64 : hh * 64 + 64, ksl],
                                qt[hh * 64 : hh * 64 + 64, base + co : base + ce],
                                start=True,
                                stop=True,
                            )
                        p_t = pt_pool.tile([128, 1024], BF16, name="p_t")
                        nc.scalar.activation(
                            p_t[:, o0:wid], sps[:, o0:wid], EXP, scale=SCALE
                        )
                        if half == 0:
                            nc.vector.tensor_mul(
                                p_t[:, o0 : o0 + 128],
                                p_t[:, o0 : o0 + 128],
                                utri_sb,
                            )
                        pts.append((p_t, base, o0, wid))
                    # P@V into per-qc accumulators (row 64 = denominator)
                    for p_t, base, o0, wid in pts:
                        for j in range(0, wid, 512):
                            co = max(o0, j)
                            ce = min(j + 512, wid)
                            if co >= ce:
                                continue
                            qc = (base + co) // 512
                            nc.tensor.matmul(
                                outT[qc][0:65, (base + co) % 512 : (base + co) % 512 + ce - co],
                                Vsb[kt][:, h, :],
                                p_t[:, co:ce],
                                start=(kt == 0),
                                stop=(kt == qc * 4 + 3),
                            )
                    # normalize the q-chunk completed by this diagonal k-tile
                    if r == 3:
                        normalize(h, qc0, outT[qc0])
                    if post is not None:
                        for thunk in post.get(kt, ()):
                            thunk()

            def phase3(qc):
                """Output projection for q-chunk qc."""
                for sti in range(4):
                    st = qc * 4 + sti
                    sl = slice(sti * 128, (sti + 1) * 128)
                    op = ps.tile([128, 1024], F32, tag="strip", bufs=2, name=f"op{st}")
                    for ec in range(2):
                        for ct in range(2):
                            nc.tensor.matmul(
                                op[:, ec * 512 : (ec + 1) * 512],
                                attnT[qc][:, ct, sl],
                                wo_sb[:, ct, ec * 512 : (ec + 1) * 512],
                                start=(ct == 0),
                                stop=(ct == 1),
                            )
                    ob = outp.tile([128, 1024], F32, name="ob")
                    if sti % 2 == 0:
                        nc.vector.tensor_copy(ob, op)
                    else:
                        nc.scalar.copy(ob, op)
                    nc.sync.dma_start(
                        out=out[st * 128 : (st + 1) * 128, 0:512], in_=ob[:, 0:512]
                    )
                    nc.gpsimd.dma_start(
                        out=out[st * 128 : (st + 1) * 128, 512:1024], in_=ob[:, 512:1024]
                    )

            # ---- schedule ----
            head0_qmajor()

            # interleave Q/K projection for heads 2-3 into head 1's k-loop
            pre1 = {
                0: [lambda: qk_chunk(1, 0)],
                1: [lambda: qk_chunk(1, 1)],
                2: [lambda: qk_chunk(1, 2)],
                3: [lambda: qk_chunk(1, 3)],
                4: [lambda: qk_chunk(3, 0)],
                5: [lambda: qk_chunk(3, 1)],
                6: [lambda: qk_chunk(3, 2)],
                7: [lambda: qk_chunk(3, 3)],
            }
            attn_head(1, pre=pre1)
            attn_head(2)
            # head 3: emit phase3(qc) right after its normalize closes qc
            post3 = {qc * 4 + 3: [lambda qc=qc: phase3(qc)] for qc in range(NSC)}
            attn_head(3, post=post3)
    nc.compile()
    return nc


def _host_tables():
    half = HD // 2
    inv_freq = 1.0 / (ROPE_BASE ** (np.arange(0, half, dtype=np.float64) / half))
    ang = np.arange(S, dtype=np.float64)[:, None] * inv_freq[None, :]  # [S, 32]
    cosT = np.cos(ang).T  # [32, S]
    sinT = np.sin(ang).T
    cos64 = np.concatenate([cosT, cosT], axis=0)  # [64, S]
    tan64 = np.concatenate([sinT / cosT, sinT / cosT], axis=0)
    cosb = np.tile(cos64, (2, 1))
    tanb = np.tile(tan64, (2, 1))

    # rot permutation (sign-folded): rot[c] = -x[c+32], rot[c+32] = x[c]
    # per 64-channel head block; PermT[r, c] so that rot = PermT.T @ x
    permT = np.zeros((128, 128), dtype=np.float64)
    for blk in range(2):
        o = blk * 64
        for c in range(32):
            permT[o + c + 32, o + c] = -1.0
            permT[o + c, o + c + 32] = 1.0

    kk = np.arange(128)[:, None]
    qq = np.arange(128)[None, :]
    utri = (qq >= kk).astype(np.float64)
    return (
        np.ascontiguousarray(cosb.astype(NP_BF16)),
        np.ascontiguousarray(tanb.astype(NP_BF16)),
        np.ascontiguousarray(permT.astype(NP_BF16)),
        np.ascontiguousarray(utri.astype(NP_BF16)),
    )


def _dtiles(w, d_in, width):
    """[d_in, width] -> [128, d_in//128, width] bf16 d-tiled."""
    t = w.reshape(d_in // 128, 128, width).transpose(1, 0, 2)
    return np.ascontiguousarray(t.astype(NP_BF16))


_NC_CACHE = None


def kernel(x, w_qkv, w_out):
    global _NC_CACHE
    x = np.asarray(x, dtype=np.float32)
    w_qkv = np.asarray(w_qkv, dtype=np.float32)
    w_out = np.asarray(w_out, dtype=np.float32)

    cosb, tanb, permb, utrib = _host_tables()
    wq = w_qkv[:, 0:D]
    wk = w_qkv[:, D : 2 * D]
    wv = w_qkv[:, 2 * D : 3 * D]

    in_maps = []
    for c in range(8):
        b, hg = c // 4, c % 4
        cols = slice(hg * CV, (hg + 1) * CV)
        xT = np.ascontiguousarray(x[b].T)  # [1024, 2048]
        # x d-tiled then split into per-sc chunks: [4, 128, 8, 512]
        xd = _dtiles(xT, D, S).reshape(128, 8, 4, 512).transpose(2, 0, 1, 3)
        wqk = np.concatenate([wq[:, cols], wk[:, cols]], axis=1)  # [1024, 512]
        wo = w_out[cols, :]  # [256, 1024]
        in_maps.append(
            {
                "xb8": np.ascontiguousarray(xd),
                "wqkb": _dtiles(wqk, D, 512),
                "wvb": _dtiles(wv[:, cols], D, 256),
                "wob": _dtiles(wo, 256, D),
                "cosb": cosb,
                "tanb": tanb,
                "permb": permb,
                "utrib": utrib,
            }
        )

    if _NC_CACHE is None:
        _NC_CACHE = _build_nc()
    res = run_bass_kernel_spmd(_NC_CACHE, in_maps, core_ids=list(range(8)))
    out = np.zeros((B, S, D), dtype=np.float32)
    for c in range(8):
        out[c // 4] += res.results[c]["out"]
    return out
